# revision 26
# baseline (speedup 1.0000x reference)
"""2-layer GCN (PyG GCNConv, bias=False, normalize=True) on 8 TRN2 NeuronCores.

Math: out = A @ relu(A @ X @ W1) @ W2 with A = D^{-1/2} (A_w + I) D^{-1/2}.

Structure (v2):
- Nodes are re-permuted into 80 bins (8 cores x 10 blocks, 125 nodes each,
  balanced by in-degree) so every (core, block) has a near-equal edge count.
- Layer 1 aggregation A@X streams a HOST-PREGATHERED tensor GX (source rows
  of X in edge-slot order, deduped per dst block) linearly from DRAM and
  PSUM-accumulates indicator matmuls: agg = sum_t IND_t^T @ GX_t. This
  removes all Pool-engine (SWDGE dma_gather) work from layer 1, which was
  the baseline bottleneck (~8.4 ns/edge of descriptor emission).
- Per block: h2 = relu(agg @ W1) @ W2 via PE transposes + weight matmuls.
- H2 shards AllGather in GROUPS (fired as soon as a group of blocks is
  done) so the collective overlaps layer-1 compute.
- Layer 2: the first N_GATHER_BLOCKS dst blocks aggregate via Pool
  dma_gather from the allgathered H2 (Pool is otherwise idle in layer 2),
  reusing the same SBUF-resident IND tiles as layer 1; remaining blocks use
  DENSE [128 src x 128 dst] indicator tiles streamed from DRAM against the
  SBUF-resident H2F tile stack (TensorE + DMA). The two paths run on
  disjoint engines and are balanced by N_GATHER_BLOCKS.
"""

import math

import numpy as np

N_CORES = 8
COMPUTE_DTYPE = "bf16"        # "f32" or "bf16"
NBLK = 10                     # dst blocks per core (128 rows each, 125 valid)
BIN_CAP = 125                 # nodes per (core, block) bin
N_GATHER_BLOCKS = 4           # layer-2 blocks aggregated via Pool dma_gather
AG_GROUPS = [4, 4, 2]         # blocks per allgather group
L1_CHUNK = 8                  # GX tiles per stream chunk
L2_CHUNK = 8                  # gather tiles per dma_gather call (<=8: 1024 idx)
D2_CHUNK = 16                 # dense ind2 tiles per stream chunk


# --------------------------------------------------------------------------
# host-side graph packing
# --------------------------------------------------------------------------
def _pack_graph(x, edge_index, edge_weight, n_nodes, n_cores, np_cdt):
    src = np.asarray(edge_index[0], dtype=np.int64)
    dst = np.asarray(edge_index[1], dtype=np.int64)
    w = np.asarray(edge_weight, dtype=np.float32)

    deg = np.zeros(n_nodes, dtype=np.float32)
    np.add.at(deg, dst, w)
    deg += np.float32(1.0)
    dinv = (1.0 / np.sqrt(deg)).astype(np.float32)
    norm = (dinv[src] * w * dinv[dst]).astype(np.float32)

    # fold self loops (coefficient 1/deg) in as ordinary edges
    iota = np.arange(n_nodes, dtype=np.int64)
    s_all = np.concatenate([src, iota])
    d_all = np.concatenate([dst, iota])
    v_all = np.concatenate([norm, (1.0 / deg).astype(np.float32)])

    nbins = n_cores * NBLK
    assert nbins * BIN_CAP == n_nodes

    # ---- balanced node -> (core, block, col) assignment by in-edge count
    indeg = np.zeros(n_nodes, dtype=np.int64)
    np.add.at(indeg, d_all, 1)
    order = np.argsort(-indeg, kind="stable")
    import heapq

    bin_load = np.zeros(nbins, dtype=np.int64)
    bin_fill = np.zeros(nbins, dtype=np.int64)
    bin_nodes = np.full((nbins, BIN_CAP), -1, dtype=np.int64)
    heap = [(0, 0, b) for b in range(nbins)]
    heapq.heapify(heap)
    for n in order:
        while True:
            _, _, b = heapq.heappop(heap)
            if bin_fill[b] < BIN_CAP:
                break
        bin_nodes[b, bin_fill[b]] = n
        bin_fill[b] += 1
        bin_load[b] += indeg[n]
        if bin_fill[b] < BIN_CAP:
            heapq.heappush(heap, (int(bin_load[b]), int(bin_fill[b]), b))
    assert (bin_fill == BIN_CAP).all()

    node_core = np.empty(n_nodes, dtype=np.int64)
    node_blk = np.empty(n_nodes, dtype=np.int64)
    node_col = np.empty(n_nodes, dtype=np.int64)
    for b in range(nbins):
        ns = bin_nodes[b]
        node_core[ns] = b // NBLK
        node_blk[ns] = b % NBLK
        node_col[ns] = np.arange(BIN_CAP)

    # ---- allgather group layout: position of node n in the gathered H2
    g_of_blk = np.empty(NBLK, dtype=np.int64)
    gb0_arr = np.empty(NBLK, dtype=np.int64)
    gnb_arr = np.empty(NBLK, dtype=np.int64)
    base = np.zeros(len(AG_GROUPS) + 1, dtype=np.int64)
    acc = 0
    for g, gnb in enumerate(AG_GROUPS):
        base[g + 1] = base[g] + n_cores * gnb * 128
        g_of_blk[acc : acc + gnb] = g
        gb0_arr[acc : acc + gnb] = acc
        gnb_arr[acc : acc + gnb] = gnb
        acc += gnb
    assert acc == NBLK

    def pos_of(nodes):
        c = node_core[nodes]
        b = node_blk[nodes]
        col = node_col[nodes]
        g = g_of_blk[b]
        return base[g] + (c * gnb_arr[b] + (b - gb0_arr[b])) * 128 + col

    n_pos = int(base[-1])
    ns_tiles = n_pos // 128                    # src tiles in H2F
    n_groups = len(AG_GROUPS)
    pos_all = pos_of(s_all)                    # per-edge source position

    # ---- per-core per-block edge lists, deduped by (block, src).
    # Slots within a block are ordered by the allgather GROUP of the source
    # position and padded to tile boundaries per group, so each layer-2
    # dma_gather call reads exactly one group's collective output (and can
    # start as soon as that group's allgather lands).
    e_core = node_core[d_all]
    e_blk = node_blk[d_all]
    e_col = node_col[d_all]

    key = (e_core * NBLK + e_blk) * n_nodes + s_all
    uniq, inv = np.unique(key, return_inverse=True)
    u_core = uniq // (NBLK * n_nodes)
    u_blk = (uniq // n_nodes) % NBLK
    u_src = uniq % n_nodes
    u_pos = pos_of(u_src)
    u_grp = np.searchsorted(base[1:], u_pos, side="right")  # group of source

    ucnt_g = np.zeros((n_cores, NBLK, n_groups), dtype=np.int64)
    np.add.at(ucnt_g, (u_core, u_blk, u_grp), 1)
    ucnt = ucnt_g.sum(axis=2)
    # shared tile structure: tiles per (block, group) = max over cores
    t_bg = np.zeros((NBLK, n_groups), dtype=np.int64)
    for b in range(NBLK):
        for g in range(n_groups):
            t_bg[b, g] = int(math.ceil(ucnt_g[:, b, g].max() / 128.0))
    t_blocks = [int(t_bg[b].sum()) for b in range(NBLK)]
    tile_off = np.concatenate([[0], np.cumsum(t_blocks)]).astype(np.int64)
    # first tile of (block, group)
    tile_off_bg = np.zeros((NBLK, n_groups), dtype=np.int64)
    for b in range(NBLK):
        o = int(tile_off[b])
        for g in range(n_groups):
            tile_off_bg[b, g] = o
            o += int(t_bg[b, g])
    tot_tiles = int(tile_off[-1])
    tot_slots = tot_tiles * 128

    # rank of each unique entry within its (core, blk, grp); sort by
    # (core, blk, grp) -- uniq is (core, blk, src)-sorted already
    sort2 = np.lexsort((u_pos, u_grp, u_blk, u_core))
    u_core, u_blk, u_src, u_pos, u_grp = (
        u_core[sort2], u_blk[sort2], u_src[sort2], u_pos[sort2], u_grp[sort2]
    )
    inv = np.argsort(sort2, kind="stable")[inv]
    ubg_key = (u_core * NBLK + u_blk) * n_groups + u_grp
    starts = np.searchsorted(ubg_key, np.arange(n_cores * NBLK * n_groups))
    rank = np.arange(len(uniq)) - starts[ubg_key]
    u_slot = tile_off_bg[u_blk, u_grp] * 128 + rank
    e_slot = u_slot[inv]

    xc = np.ascontiguousarray(np.asarray(x, dtype=np.float32).astype(np_cdt))
    f1 = xc.shape[1]

    gx_list, ind_list, idxw_list, cnt_list = [], [], [], []
    for c in range(n_cores):
        # ---- GX: [128, tot_tiles * f1], slot s -> (partition s%128, tile s//128)
        m = u_core == c
        slots_c = u_slot[m]
        src_c = u_src[m]
        rows = np.zeros((tot_slots, f1), dtype=np_cdt)
        rows[slots_c] = xc[src_c]
        gx = np.ascontiguousarray(
            rows.reshape(tot_tiles, 128, f1).transpose(1, 0, 2).reshape(128, -1)
        )
        gx_list.append(gx)

        # ---- IND: [128, tot_slots], ind[s%128, (s//128)*128 + dcol] += coef
        em = e_core == c
        es, ec, ev = e_slot[em], e_col[em], v_all[em]
        ind = np.zeros((128, tot_slots), dtype=np.float32)
        np.add.at(ind, (es % 128, (es // 128) * 128 + ec), ev)
        ind_list.append(np.ascontiguousarray(ind.astype(np_cdt)))

        # ---- layer-2 gather indices: GROUP-RELATIVE positions of the
        # unique sources (each call reads one group's collective output)
        idx_slots = np.zeros(tot_slots, dtype=np.int16)
        pos_c = u_pos[m]
        grp_c = u_grp[m]
        idx_slots[slots_c] = (pos_c - base[grp_c]).astype(np.int16)
        cnts = []
        # g-major order: matches the device's Pool-stream emission
        # ([AG g][gathers of group g for all blocks][AG g+1]...)
        for gg in range(n_groups):
            for b in range(N_GATHER_BLOCKS):
                s0 = int(tile_off_bg[b, gg]) * 128
                nslot = int(t_bg[b, gg]) * 128
                cnt = int(ucnt_g[c, b, gg])
                done = 0
                while done < nslot:
                    cl = min(L2_CHUNK * 128, nslot - done)
                    is_final = done + cl >= nslot
                    if is_final and cnt > done:
                        valid = cnt - done
                        idx_slots[s0 + done + valid : s0 + done + cl] = -1
                        cnts.append(valid)
                    elif cnt <= done:
                        # sub-run fully padded (no valid entries in chunk)
                        idx_slots[s0 + done : s0 + done + cl] = -1
                        cnts.append(0)
                    else:
                        cnts.append(cl)
                    done += cl
        cnt_list.append(np.array(cnts, dtype=np.uint32)[None, :])

        s = np.arange(tot_slots)
        idxw = np.zeros((128, tot_slots // 16), dtype=np.int16)
        idxw[s % 16, s // 16] = idx_slots
        for r in range(1, 8):
            idxw[16 * r : 16 * (r + 1)] = idxw[:16]
        idxw_list.append(idxw)

    # ---- dense layer-2 indicator tiles for blocks >= N_GATHER_BLOCKS
    n_dense = NBLK - N_GATHER_BLOCKS
    ind2_list = []
    for c in range(n_cores):
        em = (e_core == c) & (e_blk >= N_GATHER_BLOCKS)
        ep, eb, ec, ev = pos_all[em], e_blk[em], e_col[em], v_all[em]
        bi = eb - N_GATHER_BLOCKS
        ind2 = np.zeros((128, n_dense * ns_tiles * 128), dtype=np.float32)
        np.add.at(ind2, (ep % 128, (bi * ns_tiles + ep // 128) * 128 + ec), ev)
        ind2_list.append(np.ascontiguousarray(ind2.astype(np_cdt)))

    return dict(
        gx=gx_list,
        ind=ind_list,
        idxw=idxw_list,
        cnts=cnt_list,
        ind2=ind2_list,
        n_calls=len(cnt_list[0][0]),
        t_blocks=t_blocks,
        tile_off=tile_off,
        t_bg=t_bg,
        tile_off_bg=tile_off_bg,
        tot_tiles=tot_tiles,
        ns_tiles=ns_tiles,
        n_pos=n_pos,
        bin_nodes=bin_nodes,
    )


# --------------------------------------------------------------------------
# device kernel
# --------------------------------------------------------------------------
def _build_nc(f1, f2, f3, t_blocks, tile_off, n_cores, n_calls,
              ns_tiles, n_pos, t_bg, tile_off_bg, compute_dtype="bf16"):
    import concourse.mybir as mybir
    import concourse.tile as tile
    from concourse import bacc
    from concourse.masks import make_identity

    f32 = mybir.dt.float32
    i16 = mybir.dt.int16
    cdt = mybir.dt.bfloat16 if compute_dtype == "bf16" else mybir.dt.float32
    tot_tiles = int(tile_off[-1])
    tot_slots = tot_tiles * 128
    kf1, kf2 = f1 // 128, f2 // 128
    n_dense = NBLK - N_GATHER_BLOCKS
    # tiles of the gather blocks stay SBUF-resident (shared by both layers)
    gtiles = int(tile_off[N_GATHER_BLOCKS])

    nc = bacc.Bacc(num_devices=n_cores)
    gx_ext = nc.declare_dram_parameter("gx", [128, tot_tiles * f1], cdt, isOutput=False)
    ind_ext = nc.declare_dram_parameter("ind", [128, tot_slots], cdt, isOutput=False)
    w1_ext = nc.declare_dram_parameter("w1", [f1, f2], cdt, isOutput=False)
    w2_ext = nc.declare_dram_parameter("w2", [f2, f3], cdt, isOutput=False)
    idx_ext = nc.declare_dram_parameter("idxw", [128, tot_slots // 16], i16, isOutput=False)
    cnt_ext = nc.declare_dram_parameter("cnts", [1, max(1, n_calls)], mybir.dt.uint32,
                                        isOutput=False)
    if n_dense:
        ind2_ext = nc.declare_dram_parameter(
            "ind2", [128, n_dense * ns_tiles * 128], cdt, isOutput=False
        )
    out_ext = nc.declare_dram_parameter("out", [NBLK * 128, f3], f32, isOutput=True)

    with tile.TileContext(nc) as tc:
        with tc.tile_pool(name="dram", bufs=1, space="DRAM") as dpool, \
             tc.tile_pool(name="const", bufs=1) as cpool, \
             tc.tile_pool(name="gxp", bufs=3) as gxpool, \
             tc.tile_pool(name="indp", bufs=3) as indpool, \
             tc.tile_pool(name="i2p", bufs=2) as i2pool, \
             tc.tile_pool(name="gbp", bufs=15) as gbpool, \
             tc.tile_pool(name="work", bufs=2) as wpool, \
             tc.tile_pool(name="psagg", bufs=2, space="PSUM") as ps_agg_p, \
             tc.tile_pool(name="pstr", bufs=1, space="PSUM") as ps_tr_p, \
             tc.tile_pool(name="psc1", bufs=1, space="PSUM") as ps_c1_p, \
             tc.tile_pool(name="psh2", bufs=1, space="PSUM") as ps_h2_p, \
             tc.tile_pool(name="pso", bufs=2, space="PSUM") as ps_o_p:

            # ---- DRAM collective buffers (one Shared output per group: a
            # Shared DRAM tensor may only have a single writing instruction)
            cc_in_g = [
                dpool.tile([gnb * 128, f3], cdt, name=f"ccin{g}")
                for g, gnb in enumerate(AG_GROUPS)
            ]
            h2p_g = [
                dpool.tile([n_cores * gnb * 128, f3], cdt, addr_space="Shared",
                           name=f"h2p{g}")
                for g, gnb in enumerate(AG_GROUPS)
            ]
            grp_lo = [0]
            for gnb in AG_GROUPS:
                grp_lo.append(grp_lo[-1] + n_cores * gnb * 128)

            # ---- constants
            cnt_sb = cpool.tile([1, max(1, n_calls)], mybir.dt.uint32)
            nc.sync.dma_start(out=cnt_sb[:, :], in_=cnt_ext[:, :])
            idx_sb = cpool.tile([128, tot_slots // 16], i16)
            nc.sync.dma_start(out=idx_sb[:, :], in_=idx_ext[:, :])
            cnt_reg = nc.gpsimd.to_reg(0)
            call_i = [0]

            # IND tiles of the gather blocks: resident, used by both layers
            indg_sb = cpool.tile([128, gtiles * 128], cdt)
            nc.scalar.dma_start(out=indg_sb[:, :], in_=ind_ext[:, : gtiles * 128])

            w1_sb = cpool.tile([128, kf1 * f2], cdt)  # chunk (k,m) at (k*kf2+m)*128
            for k in range(kf1):
                for m_ in range(kf2):
                    nc.scalar.dma_start(
                        out=w1_sb[:, (k * kf2 + m_) * 128 : (k * kf2 + m_ + 1) * 128],
                        in_=w1_ext[k * 128 : (k + 1) * 128, m_ * 128 : (m_ + 1) * 128],
                    )
            w2_sb = cpool.tile([128, kf2 * f3], cdt)
            for k in range(kf2):
                nc.scalar.dma_start(
                    out=w2_sb[:, k * f3 : (k + 1) * f3],
                    in_=w2_ext[k * 128 : (k + 1) * 128, :],
                )
            ident = cpool.tile([128, 128], cdt)
            make_identity(nc, ident)

            h2f = cpool.tile([128, ns_tiles, f3], cdt, name="h2f")

            # ---- allgather group bookkeeping
            g_of_blk, gb0 = [], []
            acc = 0
            for g, gnb in enumerate(AG_GROUPS):
                for _ in range(gnb):
                    g_of_blk.append(g)
                    gb0.append(acc)
                acc += gnb

            def emit_ag(g):
                nc.gpsimd.collective_compute(
                    "AllGather",
                    mybir.AluOpType.bypass,
                    replica_groups=[list(range(n_cores))],
                    ins=[cc_in_g[g][:, :].opt()],
                    outs=[h2p_g[g][:, :].opt()],
                )
                # land the group into the SBUF-resident H2F tile stack
                t0 = grp_lo[g] // 128
                nt = (grp_lo[g + 1] - grp_lo[g]) // 128
                h2v = h2p_g[g][:, :].rearrange("(s p) f -> p s f", p=128)
                nc.scalar.dma_start(out=h2f[:, t0 : t0 + nt, :], in_=h2v)

            # ---------------- layer 1 ----------------
            # Software-pipelined: block b's aggregation matmuls are emitted
            # BEFORE block b-1's transform so the in-order PE stream never
            # stalls on the transform's vector/scalar steps.
            n_groups = len(AG_GROUPS)
            gb_tiles = {}           # (g, call#) -> (gb2 tile, ct, tg0, done)

            def emit_gathers(gg):
                # layer-2 gather sub-runs of group gg for all gather blocks;
                # dispatched on Pool right after AG(gg), matmuls emitted later
                for b in range(N_GATHER_BLOCKS):
                    tbg = int(t_bg[b, gg])
                    tg0 = int(tile_off_bg[b, gg])
                    done = 0
                    while done < tbg:
                        ct = min(L2_CHUNK, tbg - done)
                        gb2 = gbpool.tile([128, ct, f3], cdt, tag="gbuf")
                        cb = (tg0 + done) * 8
                        if done + ct >= tbg:
                            # final chunk of the sub-run carries the -1 index
                            # tail; skipped rows must read as finite zeros
                            nc.vector.memset(gb2[:, :ct, :], 0.0)
                        nc.gpsimd.reg_load(
                            cnt_reg, cnt_sb[0:1, call_i[0] : call_i[0] + 1]
                        )
                        call_i[0] += 1
                        nc.gpsimd.dma_gather(
                            out_ap=gb2[:, :ct, :],
                            in_ap=h2p_g[gg][:, :],
                            idxs_ap=idx_sb[:, cb : cb + ct * 8],
                            num_idxs=ct * 128,
                            num_idxs_reg=cnt_reg,
                            elem_size=f3,
                        )
                        gb_tiles.setdefault((b, gg), []).append((gb2, ct, tg0 + done))
                        done += ct

            def emit_agg(b):
                tb = t_blocks[b]
                tt0 = int(tile_off[b])
                in_resident = b < N_GATHER_BLOCKS
                ps_agg = ps_agg_p.tile([128, f1], f32, tag="agg")
                done = 0
                while done < tb:
                    ct = min(L1_CHUNK, tb - done)
                    t0 = tt0 + done
                    gxb = gxpool.tile([128, ct, f1], cdt, tag="gx")
                    nc.sync.dma_start(
                        out=gxb[:, :, :],
                        in_=gx_ext[:, t0 * f1 : (t0 + ct) * f1].rearrange(
                            "p (t f) -> p t f", t=ct
                        ),
                    )
                    if in_resident:
                        indb = indg_sb[:, t0 * 128 : (t0 + ct) * 128]
                    else:
                        indb = indpool.tile([128, ct * 128], cdt, tag="ind")
                        nc.scalar.dma_start(
                            out=indb[:, :], in_=ind_ext[:, t0 * 128 : (t0 + ct) * 128]
                        )
                    for t in range(ct):
                        tt = t0 + t
                        nc.tensor.matmul(
                            ps_agg[:, :],
                            lhsT=indb[:, t * 128 : (t + 1) * 128],
                            rhs=gxb[:, t, :],
                            start=(tt == tt0),
                            stop=(tt == tt0 + tb - 1),
                        )
                    done += ct
                return ps_agg

            def emit_xform(b, ps_agg):
                # h2 = relu(agg @ W1) @ W2, then stage into the collective in
                agg_sb = wpool.tile([128, f1], cdt, tag="agg_sb")
                nc.vector.tensor_copy(agg_sb[:, :], ps_agg[:, :])
                ps_tr = ps_tr_p.tile([128, f1], cdt, tag="tr")
                for k in range(kf1):
                    nc.tensor.transpose(
                        ps_tr[:, k * 128 : (k + 1) * 128],
                        agg_sb[:, k * 128 : (k + 1) * 128],
                        ident,
                    )
                aggT_sb = wpool.tile([128, f1], cdt, tag="aggT")
                nc.vector.tensor_copy(aggT_sb[:, :], ps_tr[:, :])

                ps_c1 = ps_c1_p.tile([128, f2], f32, tag="c1")
                firstmm = True
                for m_ in range(kf2):
                    for k in range(kf1):
                        nc.tensor.matmul(
                            ps_c1[:, m_ * 128 : (m_ + 1) * 128],
                            lhsT=w1_sb[:, (k * kf2 + m_) * 128 : (k * kf2 + m_ + 1) * 128],
                            rhs=aggT_sb[:, k * 128 : (k + 1) * 128],
                            start=firstmm,
                            stop=(m_ == kf2 - 1 and k == kf1 - 1),
                        )
                        firstmm = False
                h1T_sb = wpool.tile([128, f2], cdt, tag="h1T")
                nc.scalar.activation(
                    h1T_sb[:, :], ps_c1[:, :], mybir.ActivationFunctionType.Relu
                )
                ps_h2 = ps_h2_p.tile([128, f3], f32, tag="h2")
                for k in range(kf2):
                    nc.tensor.matmul(
                        ps_h2[:, :],
                        lhsT=h1T_sb[:, k * 128 : (k + 1) * 128],
                        rhs=w2_sb[:, k * f3 : (k + 1) * f3],
                        start=(k == 0),
                        stop=(k == kf2 - 1),
                    )
                h2_sb = wpool.tile([128, f3], cdt, tag="h2sb")
                nc.scalar.copy(h2_sb[:, :], ps_h2[:, :])
                g = g_of_blk[b]
                off = (b - gb0[b]) * 128
                nc.sync.dma_start(
                    out=cc_in_g[g][off : off + 128, :], in_=h2_sb[:, :]
                )
                if b == gb0[b] + AG_GROUPS[g] - 1:
                    emit_ag(g)
                    emit_gathers(g)

            pending = None
            for b in range(NBLK):
                ps_agg = emit_agg(b)
                if pending is not None:
                    emit_xform(*pending)
                pending = (b, ps_agg)
            emit_xform(*pending)

            # ---------------- layer 2 ----------------
            # GROUP-MAJOR over the allgather groups so the in-order PE stream
            # never stalls on a later allgather. Per-block partials accumulate
            # in SBUF f32. Gather-block matmuls for group g are emitted one
            # dense group later, giving Pool time to finish group g's gathers.
            grp_t0 = [grp_lo[g] // 128 for g in range(n_groups)]
            grp_t1 = [grp_lo[g + 1] // 128 for g in range(n_groups)]
            acc_sb = {}
            for b in range(NBLK):
                acc_sb[b] = cpool.tile([128, f3], f32, name=f"acc{b}")

            def acc_update(b, ps_o, first, last):
                if first:
                    nc.vector.tensor_copy(acc_sb[b][:, :], ps_o[:, :])
                elif not last:
                    nc.vector.tensor_tensor(
                        out=acc_sb[b][:, :], in0=acc_sb[b][:, :],
                        in1=ps_o[:, :], op=mybir.AluOpType.add,
                    )
                else:
                    o_sb = wpool.tile([128, f3], f32, tag="osb")
                    nc.vector.tensor_tensor(
                        out=o_sb[:, :], in0=acc_sb[b][:, :],
                        in1=ps_o[:, :], op=mybir.AluOpType.add,
                    )
                    nc.sync.dma_start(
                        out=out_ext[b * 128 : (b + 1) * 128, :], in_=o_sb[:, :]
                    )

            def emit_dense_group(gg):
                s0, s1 = grp_t0[gg], grp_t1[gg]
                for bi in range(n_dense):
                    b = N_GATHER_BLOCKS + bi
                    done = s0
                    ps_o = ps_o_p.tile([128, f3], f32, tag="o")
                    while done < s1:
                        ct = min(D2_CHUNK, s1 - done)
                        i2 = i2pool.tile([128, ct * 128], cdt, tag="i2")
                        nc.scalar.dma_start(
                            out=i2[:, :],
                            in_=ind2_ext[
                                :, (bi * ns_tiles + done) * 128
                                : (bi * ns_tiles + done + ct) * 128
                            ],
                        )
                        for s in range(ct):
                            nc.tensor.matmul(
                                ps_o[:, :],
                                lhsT=i2[:, s * 128 : (s + 1) * 128],
                                rhs=h2f[:, done + s, :],
                                start=(done + s == s0),
                                stop=(done + s == s1 - 1),
                            )
                        done += ct
                    acc_update(b, ps_o, gg == 0, gg == n_groups - 1)

            def emit_gather_mms(gg):
                for b in range(N_GATHER_BLOCKS):
                    runs = gb_tiles.get((b, gg), [])
                    if not runs:
                        if gg == 0:
                            nc.vector.memset(acc_sb[b][:, :], 0.0)
                        continue
                    last_t = runs[-1][2] + runs[-1][1] - 1
                    ps_o = ps_o_p.tile([128, f3], f32, tag="o")
                    for gb2, ct, tstart in runs:
                        for t in range(ct):
                            tt = tstart + t
                            nc.tensor.matmul(
                                ps_o[:, :],
                                lhsT=indg_sb[:, tt * 128 : (tt + 1) * 128],
                                rhs=gb2[:, t, :],
                                start=(tt == runs[0][2]),
                                stop=(tt == last_t),
                            )
                    acc_update(b, ps_o, gg == 0, gg == n_groups - 1)

            for gg in range(n_groups):
                emit_dense_group(gg)
                if gg >= 1:
                    emit_gather_mms(gg - 1)
            emit_gather_mms(n_groups - 1)

    nc.finalize()
    return nc


# --------------------------------------------------------------------------
# top level
# --------------------------------------------------------------------------
def build_all(x, edge_index, edge_weight, W1, W2, n_cores=N_CORES,
              compute_dtype=COMPUTE_DTYPE):
    if compute_dtype == "bf16":
        import ml_dtypes

        np_cdt = ml_dtypes.bfloat16
    else:
        np_cdt = np.float32
    W1c = np.ascontiguousarray(np.asarray(W1, dtype=np.float32).astype(np_cdt))
    W2c = np.ascontiguousarray(np.asarray(W2, dtype=np.float32).astype(np_cdt))
    n_nodes = np.asarray(x).shape[0]
    f1, f2, f3 = W1c.shape[0], W1c.shape[1], W2c.shape[1]
    g = _pack_graph(x, edge_index, edge_weight, n_nodes, n_cores, np_cdt)
    nc = _build_nc(
        f1, f2, f3, g["t_blocks"], g["tile_off"], n_cores, g["n_calls"],
        g["ns_tiles"], g["n_pos"], g["t_bg"], g["tile_off_bg"],
        compute_dtype=compute_dtype,
    )
    in_maps = []
    for c in range(n_cores):
        in_maps.append({
            "gx": g["gx"][c],
            "ind": g["ind"][c],
            "w1": W1c,
            "w2": W2c,
            "idxw": g["idxw"][c],
            "cnts": g["cnts"][c],
            "ind2": g["ind2"][c],
        })
    return nc, in_maps, g


def _unpermute(res, g, n_nodes, f3, n_cores):
    out = np.empty((n_nodes, f3), dtype=np.float32)
    bin_nodes = g["bin_nodes"]
    for c in range(n_cores):
        oc = np.asarray(res[c])            # [NBLK*128, f3]
        for b in range(NBLK):
            nodes = bin_nodes[c * NBLK + b]
            out[nodes] = oc[b * 128 : b * 128 + BIN_CAP]
    return out


def kernel(x, edge_index, edge_weight, W1, W2):
    from concourse.bass_utils import run_bass_kernel_spmd

    nc, in_maps, g = build_all(x, edge_index, edge_weight, W1, W2)
    res = run_bass_kernel_spmd(nc, in_maps, list(range(N_CORES)))
    outs = [res.results[c]["out"] for c in range(N_CORES)]
    return _unpermute(outs, g, np.asarray(x).shape[0], outs[0].shape[1], N_CORES)
